# revision 1
# baseline (speedup 1.0000x reference)
"""Trainium2 Bass kernel for nn_CNN_RNN_88347477278730.

Pipeline (data-parallel over batch, 8 rows per core on 8 cores):
  kernel1 (device): input projection emb @ Wih_c.T (+biases) hoisted, then the
      512-step select-policy GRUCell recurrence; per step the Gumbel-perturbed
      logit-diff decision k_t = (h . wdiff > -cdiff_t) is emitted.
  host: compaction (gather kept tokens to the front), new_lens, Ldyn, masks.
  kernel2 (device): proj of compacted embeddings, 2-layer GRU recurrences,
      valid-masking, Kim-CNN convs as shifted matmuls, masked max-pool, final
      linear.

All matmul layouts are "weights stationary": lhsT = weight tiles
[K=128, M=128], moving operand = transposed activations [K, batch], so
gate tensors land partition-major ([128 gate dims, small free]) where the
elementwise engines are fast.
"""

import os
import subprocess
import sys
import tempfile

import numpy as np

# ---------------------------------------------------------------- constants
B, T, E, H, NF = 64, 512, 768, 256, 100
NCORES = 8
BPC = B // NCORES  # batch rows per core
KE = E // 128      # 6 K-tiles over the embedding dim
KH = H // 128      # 2 K-tiles over the hidden dim
GC = (3 * H) // 128  # 6 gate chunks (r: 0-1, z: 2-3, n: 4-5)
NEG = -1.0e30

_F32 = None  # set lazily to mybir.dt.float32


# ------------------------------------------------------------- tile patch
def _apply_tile_patch():
    """This walrus build rejects >2 sem waits on one SP control instruction;
    split the TileContext tail drain into several drains of <=2 waits."""
    import concourse.tile as tile
    from concourse.vector_clock import ScopedClock, VectorClock

    if getattr(tile.TileContext, "_drain_split_patched", False):
        return

    def _patched(self, tick_clock, wait_clock):
        gc = tick_clock.global_clock
        n = len(gc)
        for start in range(0, n, 1):
            vec = [0] * n
            any_set = False
            for p in range(start, min(start + 1, n)):
                vec[p] = gc[p]
                any_set = any_set or vec[p] > 0
            if not any_set:
                continue
            d = self.nc.sync.drain()
            wait_clock.add_sem_waits(d.ins, ScopedClock({None: VectorClock(vec)}))
        self.nc.all_engine_barrier()
        assert self.sems is not None
        popped = self.nc._tile_sem_poison_stack.pop()
        assert popped is self._sem_poison
        self.nc.clear_and_free_semaphores(list(self.sems.allocated().values()))
        self.nc.all_engine_barrier()

    tile.TileContext._drain_and_barrier = _patched
    tile.TileContext._drain_split_patched = True


# ------------------------------------------------------------- gumbel (CPU)
def _gumbel_cpu():
    """jax.random.gumbel(key(42), (T-1, B, 2), f32) — computed in a CPU-jax
    subprocess so the accelerator backend is never involved (it must be
    bit-identical to the reference's CPU computation)."""
    path = os.path.join(tempfile.mkdtemp(), "gumbel.npy")
    code = (
        "import numpy as np, jax, jax.numpy as jnp\n"
        f"g = jax.random.gumbel(jax.random.key(42), ({T - 1}, {B}, 2), jnp.float32)\n"
        f"np.save({path!r}, np.asarray(g))\n"
    )
    env = dict(os.environ)
    env["TRN_TERMINAL_POOL_IPS"] = ""
    env["JAX_PLATFORMS"] = "cpu"
    extra = [p for p in sys.path if p and os.path.isdir(p)]
    env["PYTHONPATH"] = os.pathsep.join(extra)
    subprocess.run([sys.executable, "-c", code], env=env, check=True, capture_output=True)
    return np.load(path)


# ------------------------------------------------------------- host packing
def _pack_T(a2d):
    """[rows(=128*k), cols] -> [128, k, cols] weight-tile layout."""
    rows, cols = a2d.shape
    k = rows // 128
    return np.ascontiguousarray(a2d.reshape(k, 128, cols).transpose(1, 0, 2)).astype(np.float32)


def _pack_bias(b1d):
    """[128*k] -> [128, k]"""
    k = b1d.shape[0] // 128
    return np.ascontiguousarray(b1d.reshape(k, 128).T).astype(np.float32)


def _pack_embT(emb_rows, t_len=T):
    """[bpc, T, E] -> [KE, 128, bpc*T] (e-major tiles, free dims (b, t))."""
    bpc = emb_rows.shape[0]
    x = emb_rows.transpose(2, 0, 1).reshape(KE, 128, bpc * t_len)
    return np.ascontiguousarray(x).astype(np.float32)


def _pack_gru_weights(Wih, Whh, bih, bhh, extra_col=None):
    """Returns (wihT, whhT, bias_proj, bhhn_rep) packings.

    bias_proj folds bih+bhh for the r,z chunks (added once at projection
    time); n chunks get bih only, with bhh_n applied per-step (it must be
    added to h@Whh_n *before* the r* multiply).
    """
    wihT = _pack_T(np.ascontiguousarray(Wih.T))  # [128, KE or KH, 3H]
    wp = np.ascontiguousarray(Whh.T)  # [H, 3H]
    if extra_col is not None:
        wp = np.concatenate([wp, extra_col[:, None]], axis=1)  # [H, 3H+1]
    whhT = _pack_T(wp)  # [128, KH, 3H(+1)]
    bias = np.empty(3 * H, np.float32)
    bias[: 2 * H] = bih[: 2 * H] + bhh[: 2 * H]
    bias[2 * H :] = bih[2 * H :]
    bias_proj = _pack_bias(bias)  # [128, GC]
    bhhn = _pack_bias(bhh[2 * H :])  # [128, KH]
    bhhn_rep = np.ascontiguousarray(
        np.broadcast_to(bhhn[:, :, None], (128, KH, BPC))
    ).astype(np.float32)
    return wihT, whhT, bias_proj, bhhn_rep


# ------------------------------------------------------------- bass builders
def _mk_nc():
    import concourse.bass as bass

    return bass.Bass("TRN2", target_bir_lowering=False, debug=False, num_devices=1)


def _split_excess_waits(nc, max_waits=1):
    """This walrus build can only encode ~2 sem waits per instruction
    (setupSyncWait 'Too many sync wait commands'). Hoist excess waits onto
    same-engine NoOps inserted just before the over-subscribed instruction;
    engine queues execute in order, so the wait semantics are identical."""
    from concourse import mybir

    nid = [0]
    for f in nc.m.functions:
        for bb in f.blocks:
            out = []
            changed = False
            for inst in bb.instructions:
                si = inst.sync_info
                lim = max_waits
                if si is not None and si.on_wait and len(si.on_wait) > lim:
                    waits = list(si.on_wait)
                    extra, keep = waits[:-lim], waits[-lim:]
                    for j in range(0, len(extra), max_waits):
                        nop = mybir.InstNoOp(
                            name=f"I-waitnop-{nid[0]}", ins=[], outs=[])
                        nid[0] += 1
                        nop.engine = inst.engine
                        nop.sync_info = mybir.SyncInfo(
                            on_wait=extra[j : j + max_waits], on_update=[])
                        nc.register_instruction(nop, overwrite=True)
                        out.append(nop)
                    inst.sync_info = mybir.SyncInfo(
                        on_wait=keep, on_update=list(si.on_update or []))
                    changed = True
                out.append(inst)
            if changed:
                bb.instructions = out
    return nc


def _gru_step(nc, mybir, t, h_rhs, gi_tile, whh_sb, bhhn_rep_sb, pools, h_out_ap,
              lg=None, t_len=T):
    """Emit one GRU step. h_rhs: AP [128, KH, BPC] of h_{t-1} (or None for
    h=0). gi_tile[:, t, c, :] holds the projected input (+bias) for step t.
    Writes h_t to h_out_ap ([128, KH, BPC]). If lg is not None it is
    (whh_lg_col_ap, lg_psum_pool, ncdiff_sb, ks_sb) for the logit decision."""
    f32 = mybir.dt.float32
    act = mybir.ActivationFunctionType
    alu = mybir.AluOpType

    rz_pool, n_pool, sb_pool = pools

    if h_rhs is not None:
        rz_ps = rz_pool.tile([128, 4, BPC], f32, tag="rzps")
        n_ps = n_pool.tile([128, KH, BPC], f32, tag="nps")
        for c in range(4):
            for k in range(KH):
                nc.tensor.matmul(
                    rz_ps[:, c, :],
                    whh_sb[:, k, c * 128 : (c + 1) * 128],
                    h_rhs[:, k, :],
                    start=(k == 0),
                    stop=(k == KH - 1),
                )
        for c in range(4, GC):
            for k in range(KH):
                nc.tensor.matmul(
                    n_ps[:, c - 4, :],
                    whh_sb[:, k, c * 128 : (c + 1) * 128],
                    h_rhs[:, k, :],
                    start=(k == 0),
                    stop=(k == KH - 1),
                )
        # rz_raw = psum + gi ; rz = sigmoid(rz_raw)
        rz = sb_pool.tile([128, 4, BPC], f32, tag="rz")
        nc.vector.tensor_tensor(rz[:], rz_ps[:], gi_tile[:, t, 0:4, :], alu.add)
        nc.scalar.activation(rz[:], rz[:], act.Sigmoid)
        # hn = psum_n + bhh_n (per-chunk partition bias)
        hn = sb_pool.tile([128, KH, BPC], f32, tag="hn")
        for k in range(KH):
            nc.scalar.activation(
                hn[:, k, :], n_ps[:, k, :], act.Identity,
                bias=bhhn_rep_sb[:, k, 0:1],
            )
    else:
        # h == 0: h@Whh == 0 -> rz = sigmoid(gi_rz); hn = bhh_n
        rz = sb_pool.tile([128, 4, BPC], f32, tag="rz")
        nc.scalar.activation(rz[:], gi_tile[:, t, 0:4, :], act.Sigmoid)
        hn = bhhn_rep_sb

    # n = tanh(gi_n + r * hn)
    tmp = sb_pool.tile([128, KH, BPC], f32, tag="tmp")
    nc.vector.tensor_tensor(tmp[:], rz[:, 0:KH, :], hn[:, :, :], alu.mult)
    nc.vector.tensor_tensor(tmp[:], tmp[:], gi_tile[:, t, 4:GC, :], alu.add)
    nn_ = sb_pool.tile([128, KH, BPC], f32, tag="nn")
    nc.scalar.activation(nn_[:], tmp[:], act.Tanh)

    # h' = n + z * (h - n)
    if h_rhs is not None:
        d = sb_pool.tile([128, KH, BPC], f32, tag="dd")
        nc.vector.tensor_tensor(d[:], h_rhs[:, :, :], nn_[:], alu.subtract)
        nc.vector.tensor_tensor(d[:], rz[:, 2:4, :], d[:], alu.mult)
        nc.vector.tensor_tensor(h_out_ap, nn_[:], d[:], alu.add)
    else:
        # h==0: h' = n - z*n = n*(1-z) ; do d = 0 - n
        d = sb_pool.tile([128, KH, BPC], f32, tag="dd")
        nc.vector.tensor_scalar(d[:], nn_[:], -1.0, None, alu.mult)
        nc.vector.tensor_tensor(d[:], rz[:, 2:4, :], d[:], alu.mult)
        nc.vector.tensor_tensor(h_out_ap, nn_[:], d[:], alu.add)

    # decision bit: k_t = (h_t . wdiff > ncdiff_t) — uses the *updated* h
    if lg is not None and t >= 1:
        whh_lg, lg_pool, ncdiff_sb, ks_sb = lg
        lg_ps = lg_pool.tile([1, BPC], f32, tag="lgps")
        for k in range(KH):
            nc.tensor.matmul(
                lg_ps[:, :],
                whh_lg[:, k, :],
                h_out_ap[:, k, :],
                start=(k == 0),
                stop=(k == KH - 1),
            )
        nc.vector.tensor_tensor(
            ks_sb[0:1, t, :], lg_ps[:, :], ncdiff_sb[0:1, t, :], alu.is_gt
        )


def _emit_proj(nc, mybir, ctx, tc, src_dram, wT_sb, bias_sb, gi_tile, kin, t_len,
               dma_pool, ps_pool):
    """gi[c*128+p, b, t] = sum_e W[e, c*128+p] * src[e, b, t] + bias.

    src_dram: DRAM [kin, 128, BPC*t_len]; wT_sb: [128, kin, 3H];
    gi_tile: [128, t_len, GC, BPC]."""
    f32 = mybir.dt.float32
    act = mybir.ActivationFunctionType
    for b in range(BPC):
        src_sb = dma_pool.tile([128, kin, t_len], f32, tag="projsrc")
        for k in range(kin):
            nc.sync.dma_start(
                src_sb[:, k, :], src_dram[k, :, b * t_len : (b + 1) * t_len]
            )
        for c in range(GC):
            ps = ps_pool.tile([128, t_len], f32, tag="projps")
            for k in range(kin):
                nc.tensor.matmul(
                    ps[:],
                    wT_sb[:, k, c * 128 : (c + 1) * 128],
                    src_sb[:, k, :],
                    start=(k == 0),
                    stop=(k == kin - 1),
                )
            nc.scalar.activation(
                gi_tile[:, :, c, b], ps[:], act.Identity, bias=bias_sb[:, c : c + 1]
            )


def build_kernel1(t_len=T, debug_h=False):
    """Select-policy kernel: proj + recurrence + decisions. Returns nc."""
    import concourse.tile as tile
    from concourse import mybir

    _apply_tile_patch()
    nc = _mk_nc()
    f32 = mybir.dt.float32
    hdbg_d = None
    if debug_h:
        hdbg_d = nc.dram_tensor(
            "hdbg", [128, t_len, KH, BPC], f32, kind="ExternalOutput").ap()
        gidbg_d = nc.dram_tensor(
            "gidbg", [128, t_len, GC, BPC], f32, kind="ExternalOutput").ap()

    embT_d = nc.dram_tensor("embT", [KE, 128, BPC * t_len], f32, kind="ExternalInput").ap()
    wihcT_d = nc.dram_tensor("wihcT", [128, KE, 3 * H], f32, kind="ExternalInput").ap()
    whhcT_d = nc.dram_tensor("whhcT", [128, KH, 3 * H + 1], f32, kind="ExternalInput").ap()
    biasc_d = nc.dram_tensor("biasc", [128, GC], f32, kind="ExternalInput").ap()
    bhhnc_d = nc.dram_tensor("bhhnc", [128, KH, BPC], f32, kind="ExternalInput").ap()
    ncdiff_d = nc.dram_tensor("ncdiff", [t_len, BPC], f32, kind="ExternalInput").ap()
    ks_d = nc.dram_tensor("ks", [t_len, BPC], f32, kind="ExternalOutput").ap()

    with tile.TileContext(nc) as tc:
        from contextlib import ExitStack

        with ExitStack() as ctx:
            wpool = ctx.enter_context(tc.tile_pool(name="weights", bufs=1))
            gipool = ctx.enter_context(tc.tile_pool(name="gi", bufs=1))
            dma_pool = ctx.enter_context(tc.tile_pool(name="dma", bufs=1 if debug_h else 2))
            ps_pool = ctx.enter_context(tc.tile_pool(name="projps", bufs=2, space="PSUM"))
            rz_pool = ctx.enter_context(tc.tile_pool(name="rzps", bufs=2, space="PSUM"))
            n_pool = ctx.enter_context(tc.tile_pool(name="nps", bufs=2, space="PSUM"))
            lg_pool = ctx.enter_context(tc.tile_pool(name="lgps", bufs=2, space="PSUM"))
            sb_pool = ctx.enter_context(tc.tile_pool(name="gates", bufs=2))
            h_pool = ctx.enter_context(tc.tile_pool(name="hstate", bufs=2))
            misc = ctx.enter_context(tc.tile_pool(name="misc", bufs=1))

            wihcT = wpool.tile([128, KE, 3 * H], f32, tag="wihcT")
            nc.sync.dma_start(wihcT[:], wihcT_d[:])
            whhcT = wpool.tile([128, KH, 3 * H + 1], f32, tag="whhcT")
            nc.sync.dma_start(whhcT[:], whhcT_d[:])
            biasc = wpool.tile([128, GC], f32, tag="biasc")
            nc.sync.dma_start(biasc[:], biasc_d[:])
            bhhnc = wpool.tile([128, KH, BPC], f32, tag="bhhnc")
            nc.sync.dma_start(bhhnc[:], bhhnc_d[:])
            ncdiff = misc.tile([1, t_len, BPC], f32, tag="ncdiff")
            nc.sync.dma_start(ncdiff[:], ncdiff_d[:])
            ks_sb = misc.tile([1, t_len, BPC], f32, tag="kssb")
            nc.vector.memset(ks_sb[:], 0.0)

            giT = gipool.tile([128, t_len, GC, BPC], f32, tag="gi")
            _emit_proj(nc, mybir, ctx, tc, embT_d, wihcT, biasc, giT, KE, t_len,
                       dma_pool, ps_pool)

            pools = (rz_pool, n_pool, sb_pool)
            whh_lg = whhcT[:, :, 3 * H : 3 * H + 1]
            lg_args = (whh_lg, lg_pool, ncdiff, ks_sb)

            hdbg_sb = None
            if debug_h:
                hdbg_sb = gipool.tile([128, t_len, KH, BPC], f32, tag="hdbg")

            h_prev = None
            for t in range(t_len):
                if debug_h:
                    h_out = hdbg_sb[:, t, :, :]
                else:
                    h_new = h_pool.tile([128, KH, BPC], f32, tag="h")
                    h_out = h_new[:, :, :]
                _gru_step(nc, mybir, t, h_prev, giT, whhcT, bhhnc, pools,
                          h_out, lg=lg_args, t_len=t_len)
                h_prev = h_out

            nc.sync.dma_start(ks_d[:], ks_sb[:])
            if debug_h:
                nc.sync.dma_start(hdbg_d[:], hdbg_sb[:])
                nc.sync.dma_start(gidbg_d[:], giT[:])

    return _split_excess_waits(nc)


def build_kernel2(t_len=T):
    """GRU0/GRU1 + convs + pooling + final linear. Returns nc."""
    import concourse.tile as tile
    from concourse import mybir

    _apply_tile_patch()
    nc = _mk_nc()
    f32 = mybir.dt.float32
    act = mybir.ActivationFunctionType
    alu = mybir.AluOpType

    nembT_d = nc.dram_tensor("nembT", [KE, 128, BPC * t_len], f32, kind="ExternalInput").ap()
    wih0T_d = nc.dram_tensor("wih0T", [128, KE, 3 * H], f32, kind="ExternalInput").ap()
    whh0T_d = nc.dram_tensor("whh0T", [128, KH, 3 * H], f32, kind="ExternalInput").ap()
    bias0_d = nc.dram_tensor("bias0", [128, GC], f32, kind="ExternalInput").ap()
    bhhn0_d = nc.dram_tensor("bhhn0", [128, KH, BPC], f32, kind="ExternalInput").ap()
    wih1T_d = nc.dram_tensor("wih1T", [128, KH, 3 * H], f32, kind="ExternalInput").ap()
    whh1T_d = nc.dram_tensor("whh1T", [128, KH, 3 * H], f32, kind="ExternalInput").ap()
    bias1_d = nc.dram_tensor("bias1", [128, GC], f32, kind="ExternalInput").ap()
    bhhn1_d = nc.dram_tensor("bhhn1", [128, KH, BPC], f32, kind="ExternalInput").ap()
    vt_d = nc.dram_tensor("vt", [1, BPC * t_len], f32, kind="ExternalInput").ap()
    cw_d = nc.dram_tensor("cw", [128, 12, KH, NF], f32, kind="ExternalInput").ap()
    cb_d = nc.dram_tensor("cb", [NF, 3], f32, kind="ExternalInput").ap()
    tmask_d = nc.dram_tensor("tmask", [NF, 3, t_len], f32, kind="ExternalInput").ap()
    woutT_d = nc.dram_tensor("woutT", [NF, 3], f32, kind="ExternalInput").ap()
    bout_d = nc.dram_tensor("bout", [1, 1], f32, kind="ExternalInput").ap()
    out_d = nc.dram_tensor("out", [1, BPC], f32, kind="ExternalOutput").ap()

    FS = (3, 4, 5)

    with tile.TileContext(nc) as tc:
        from contextlib import ExitStack

        with ExitStack() as ctx:
            wpool = ctx.enter_context(tc.tile_pool(name="weights", bufs=1))
            gipool = ctx.enter_context(tc.tile_pool(name="gi", bufs=1))
            opool = ctx.enter_context(tc.tile_pool(name="obuf", bufs=1))
            dma_pool = ctx.enter_context(tc.tile_pool(name="dma", bufs=2))
            big_ps = ctx.enter_context(tc.tile_pool(name="bigps", bufs=2, space="PSUM"))
            rz_pool = ctx.enter_context(tc.tile_pool(name="rzps", bufs=2, space="PSUM"))
            n_pool = ctx.enter_context(tc.tile_pool(name="nps", bufs=2, space="PSUM"))
            fin_ps = ctx.enter_context(tc.tile_pool(name="finps", bufs=1, space="PSUM"))
            sb_pool = ctx.enter_context(tc.tile_pool(name="gates", bufs=2))
            misc = ctx.enter_context(tc.tile_pool(name="misc", bufs=1))

            def _load(pool, dram, shape, tag):
                t_ = pool.tile(shape, f32, tag=tag)
                nc.sync.dma_start(t_[:], dram[:])
                return t_

            # "bigw" and "whh" tags are sequentially reused slots:
            # wih0T -> wih1T -> cw, and whh0T -> whh1T. Each next tile's
            # allocation waits for the previous one's last reader.
            wih0T = _load(wpool, wih0T_d, [128, KE, 3 * H], "bigw")
            whh0T = _load(wpool, whh0T_d, [128, KH, 3 * H], "whh")
            bias0 = _load(wpool, bias0_d, [128, GC], "bias0")
            bhhn0 = _load(wpool, bhhn0_d, [128, KH, BPC], "bhhn0")
            bias1 = _load(wpool, bias1_d, [128, GC], "bias1")
            bhhn1 = _load(wpool, bhhn1_d, [128, KH, BPC], "bhhn1")
            cb = _load(misc, cb_d, [NF, 3], "cb")
            tmask = _load(misc, tmask_d, [NF, 3, t_len], "tmask")
            woutT = _load(misc, woutT_d, [NF, 3], "woutT")
            bout = _load(misc, bout_d, [1, 1], "bout")
            vt = _load(misc, vt_d, [1, BPC * t_len], "vt")

            pools = (rz_pool, n_pool, sb_pool)

            # ---- layer 0 ----
            gi0 = gipool.tile([128, t_len, GC, BPC], f32, tag="gi")
            _emit_proj(nc, mybir, ctx, tc, nembT_d, wih0T, bias0, gi0, KE, t_len,
                       dma_pool, big_ps)
            o1 = opool.tile([128, KH, BPC, t_len], f32, tag="o")
            h_prev = None
            for t in range(t_len):
                _gru_step(nc, mybir, t, h_prev, gi0, whh0T, bhhn0, pools,
                          o1[:, :, :, t], t_len=t_len)
                h_prev = o1[:, :, :, t]

            # ---- proj1: gi1 = (o1 @ Wih1.T) + bias1 ----
            wih1T = _load(wpool, wih1T_d, [128, KH, 3 * H], "bigw")
            gi1 = gipool.tile([128, t_len, GC, BPC], f32, tag="gi")
            for b in range(BPC):
                for c in range(GC):
                    ps = big_ps.tile([128, t_len], f32, tag="projps")
                    for k in range(KH):
                        nc.tensor.matmul(
                            ps[:],
                            wih1T[:, k, c * 128 : (c + 1) * 128],
                            o1[:, k, b, :],
                            start=(k == 0),
                            stop=(k == KH - 1),
                        )
                    nc.scalar.activation(
                        gi1[:, :, c, b], ps[:], act.Identity, bias=bias1[:, c : c + 1]
                    )

            # ---- layer 1 ----
            whh1T = _load(wpool, whh1T_d, [128, KH, 3 * H], "whh")
            o2 = opool.tile([128, KH, BPC, t_len], f32, tag="o")
            h_prev = None
            for t in range(t_len):
                _gru_step(nc, mybir, t, h_prev, gi1, whh1T, bhhn1, pools,
                          o2[:, :, :, t], t_len=t_len)
                h_prev = o2[:, :, :, t]

            # ---- zero o2 past new_lens: o2 *= vt ----
            # partition-broadcast vt via a K=1 ones-matmul (PE outer product)
            ones_sb = misc.tile([1, 128], f32, tag="ones")
            nc.vector.memset(ones_sb[:], 1.0)
            for b in range(BPC):
                vtb = big_ps.tile([128, t_len], f32, tag="projps")
                nc.tensor.matmul(
                    vtb[:], ones_sb[:], vt[0:1, b * t_len : (b + 1) * t_len],
                    start=True, stop=True,
                )
                for k in range(KH):
                    nc.vector.tensor_tensor(
                        o2[:, k, b, :], o2[:, k, b, :], vtb[:], alu.mult
                    )

            # ---- convs + relu + tmask + max-pool ----
            cw = _load(wpool, cw_d, [128, 12, KH, NF], "bigw")
            pooled = misc.tile([NF, 3, BPC], f32, tag="pooled")
            for b in range(BPC):
                for fi, fs in enumerate(FS):
                    nw = t_len - fs + 1
                    ps = big_ps.tile([NF, t_len], f32, tag="projps")
                    m0 = sum(FS[:fi])  # flat (fs,dt) base index
                    first = True
                    for dt in range(fs):
                        for k in range(KH):
                            nc.tensor.matmul(
                                ps[:, :nw],
                                cw[:, m0 + dt, k, :],
                                o2[:, k, b, dt : dt + nw],
                                start=first,
                                stop=(dt == fs - 1 and k == KH - 1),
                            )
                            first = False
                    crelu = sb_pool.tile([NF, t_len], f32, tag="crelu")
                    nc.scalar.activation(
                        crelu[:, :nw], ps[:, :nw], act.Relu, bias=cb[:, fi : fi + 1]
                    )
                    nc.vector.tensor_tensor(
                        crelu[:, :nw], crelu[:, :nw], tmask[:, fi, :nw], alu.add
                    )
                    nc.vector.tensor_reduce(
                        pooled[:, fi, b : b + 1], crelu[:, :nw], mybir.AxisListType.X, alu.max
                    )

            # ---- final linear ----
            fps = fin_ps.tile([1, BPC], f32)
            for fi in range(3):
                nc.tensor.matmul(
                    fps[:],
                    woutT[:, fi : fi + 1],
                    pooled[:, fi, :],
                    start=(fi == 0),
                    stop=(fi == 2),
                )
            out_sb = misc.tile([1, BPC], f32, tag="outsb")
            nc.scalar.activation(out_sb[:], fps[:], act.Identity, bias=bout[0:1, 0:1])
            nc.sync.dma_start(out_d[:], out_sb[:])

    return _split_excess_waits(nc)


# ------------------------------------------------------------- host orchestration
def _host_pack_k1(inputs, gumbel, t_len=T):
    emb = np.asarray(inputs["embedded"], np.float32)
    mask = np.asarray(inputs["mask"])
    lens = mask.sum(1)
    maxlen = int(lens.max())

    wihcT, whhcT_nolg, biasc, bhhnc = _pack_gru_weights(
        inputs["Wih_c"], inputs["Whh_c"], inputs["bih_c"], inputs["bhh_c"],
        extra_col=(inputs["Wsel"][1] - inputs["Wsel"][0]).astype(np.float32),
    )
    bdiff = float(inputs["bsel"][1] - inputs["bsel"][0])

    # ncdiff[t, b]: k_t = (h.wdiff > ncdiff); forced to 0 when t >= maxlen-1
    ncdiff = np.full((t_len, B), 1.0e30, np.float32)
    upto = min(maxlen - 1, t_len)
    for t in range(1, upto):
        ncdiff[t] = -(bdiff + gumbel[t - 1, :, 1] - gumbel[t - 1, :, 0])

    in_maps = []
    for c in range(NCORES):
        rows = slice(c * BPC, (c + 1) * BPC)
        in_maps.append({
            "embT": _pack_embT(emb[rows, :t_len], t_len),
            "wihcT": wihcT,
            "whhcT": whhcT_nolg,
            "biasc": biasc,
            "bhhnc": bhhnc,
            "ncdiff": np.ascontiguousarray(ncdiff[:, rows]),
        })
    return in_maps, lens, maxlen


def _host_compact(inputs, ks_full, lens, maxlen, t_len=T):
    """ks_full: [B, t_len] decision bits (row t=0 ignored; selected[:,0]=1)."""
    emb = np.asarray(inputs["embedded"], np.float32)
    selected = np.zeros((B, t_len), np.int64)
    selected[:, 0] = 1
    selected[:, 1:] = ks_full[:, 1:]
    pos = np.arange(t_len)
    sel_valid = np.where(pos[None, :] < (lens - 1)[:, None], selected, 0)
    new_mask = np.where(pos[None, :] == (lens - 1)[:, None], 1, sel_valid)
    new_lens = new_mask.sum(1)
    Ldyn = max(int(new_lens.max()), 7)

    new_emb = np.zeros((B, t_len, E), np.float32)
    for b in range(B):
        idx = np.nonzero(new_mask[b])[0]
        new_emb[b, : len(idx)] = emb[b, idx]
    return new_emb, new_lens, Ldyn


def _host_pack_k2(inputs, new_emb, new_lens, Ldyn, t_len=T):
    wih0T, whh0T, bias0, bhhn0 = _pack_gru_weights(
        inputs["Wih0"], inputs["Whh0"], inputs["bih0"], inputs["bhh0"])
    wih1T, whh1T, bias1, bhhn1 = _pack_gru_weights(
        inputs["Wih1"], inputs["Whh1"], inputs["bih1"], inputs["bhh1"])

    FS = (3, 4, 5)
    cw = np.zeros((128, 12, KH, NF), np.float32)
    cb = np.zeros((NF, 3), np.float32)
    m = 0
    for fi, fs in enumerate(FS):
        w = np.asarray(inputs[f"conv_w{fs}"], np.float32)  # [NF,1,fs,H]
        cb[:, fi] = np.asarray(inputs[f"conv_b{fs}"], np.float32)
        for dt in range(fs):
            wt = w[:, 0, dt, :].T  # [H, NF]
            cw[:, m, :, :] = wt.reshape(KH, 128, NF).transpose(1, 0, 2)
            m += 1

    tmask = np.full((NF, 3, t_len), NEG, np.float32)
    for fi, fs in enumerate(FS):
        kf = min(Ldyn - fs + 1, t_len - fs + 1)
        if kf > 0:
            tmask[:, fi, :kf] = 0.0

    woutT = np.ascontiguousarray(
        np.asarray(inputs["Wout"], np.float32)[0].reshape(3, NF).T)
    bout = np.asarray(inputs["bout"], np.float32).reshape(1, 1)

    vt_full = (np.arange(t_len)[None, :] < new_lens[:, None]).astype(np.float32)

    in_maps = []
    for c in range(NCORES):
        rows = slice(c * BPC, (c + 1) * BPC)
        in_maps.append({
            "nembT": _pack_embT(new_emb[rows, :t_len], t_len),
            "wih0T": wih0T, "whh0T": whh0T, "bias0": bias0, "bhhn0": bhhn0,
            "wih1T": wih1T, "whh1T": whh1T, "bias1": bias1, "bhhn1": bhhn1,
            "vt": np.ascontiguousarray(vt_full[rows].reshape(1, BPC * t_len)),
            "cw": cw, "cb": cb, "tmask": tmask, "woutT": woutT, "bout": bout,
        })
    return in_maps


_NC_CACHE = {}


def _get_nc(which, t_len=T):
    key = (which, t_len)
    if key not in _NC_CACHE:
        _NC_CACHE[key] = build_kernel1(t_len) if which == 1 else build_kernel2(t_len)
    return _NC_CACHE[key]


TRACE = False  # set True (with an NTFF hook registered) to collect exec times
LAST_STATS = {}


def kernel(**inputs):
    from concourse import bass_utils

    gumbel = _gumbel_cpu()
    core_ids = list(range(NCORES))

    in_maps1, lens, maxlen = _host_pack_k1(inputs, gumbel)
    nc1 = _get_nc(1)
    res1 = bass_utils.run_bass_kernel_spmd(nc1, in_maps1, core_ids, trace=TRACE)
    ks_full = np.concatenate([res1.results[c]["ks"].T for c in range(NCORES)], axis=0)

    new_emb, new_lens, Ldyn = _host_compact(inputs, ks_full, lens, maxlen)
    in_maps2 = _host_pack_k2(inputs, new_emb, new_lens, Ldyn)
    nc2 = _get_nc(2)
    res2 = bass_utils.run_bass_kernel_spmd(nc2, in_maps2, core_ids, trace=TRACE)
    out = np.concatenate([res2.results[c]["out"][0] for c in range(NCORES)], axis=0)
    LAST_STATS["k1_ns"] = res1.exec_time_ns
    LAST_STATS["k2_ns"] = res2.exec_time_ns
    LAST_STATS["ks"] = ks_full
    LAST_STATS["new_lens"] = new_lens
    return out.astype(np.float32)



# revision 10
# speedup vs baseline: 2.1987x; 2.1987x over previous
"""Trainium2 Bass kernel for nn_CNN_RNN_88347477278730.

Pipeline (data-parallel over batch, 8 rows per core on 8 cores):
  kernel1 (device, fp32 — decision bits are margin-sensitive, bf16 flips
      them): input projection hoisted, 512-step select-policy GRUCell
      recurrence writing h_t into an SBUF history; the Gumbel logit-diff
      decisions are batched matmuls over 64-step blocks of the history.
  host: compaction (gather kept tokens to the front), new_lens, Ldyn.
  kernel2 (device, bf16 matmuls / fp32 gates): truncated to
      TL = pad(Ldyn) timesteps (positions beyond Ldyn are masked out of
      the max-pool in the reference, so truncation is exact); 2-layer GRU,
      Kim-CNN convs as shifted matmuls, per-row window masks fold the
      packed-sequence zeroing into the pool mask, final linear.

Matmul layouts are weights-stationary: lhsT = weight tiles [K=128, M=128],
moving operand = activations [K, small], so gates land partition-major.
"""

import os
import subprocess
import sys
import tempfile

import numpy as np
import ml_dtypes

# ---------------------------------------------------------------- constants
B, T, E, H, NF = 64, 512, 768, 256, 100
NCORES = 8
BPC = B // NCORES  # batch rows per core
KE = E // 128      # 6 K-tiles over the embedding dim
KH = H // 128      # 2 K-tiles over the hidden dim
GC = (3 * H) // 128  # 6 gate chunks (r: 0-1, z: 2-3, n: 4-5)
NEG = -1.0e30
LG_BLK = 64        # decision matmul block (timesteps per batched lg matmul)

BF16 = ml_dtypes.bfloat16


# ------------------------------------------------------------- tile patch
def _apply_tile_patch():
    """This walrus build rejects >2 sem waits on one SP control instruction;
    split the TileContext tail drain into several drains of <=2 waits."""
    import concourse.tile as tile
    from concourse.vector_clock import ScopedClock, VectorClock

    if getattr(tile.TileContext, "_drain_split_patched", False):
        return

    def _patched(self, tick_clock, wait_clock):
        gc = tick_clock.global_clock
        n = len(gc)
        for start in range(0, n, 1):
            vec = [0] * n
            any_set = False
            for p in range(start, min(start + 1, n)):
                vec[p] = gc[p]
                any_set = any_set or vec[p] > 0
            if not any_set:
                continue
            d = self.nc.sync.drain()
            wait_clock.add_sem_waits(d.ins, ScopedClock({None: VectorClock(vec)}))
        self.nc.all_engine_barrier()
        assert self.sems is not None
        popped = self.nc._tile_sem_poison_stack.pop()
        assert popped is self._sem_poison
        self.nc.clear_and_free_semaphores(list(self.sems.allocated().values()))
        self.nc.all_engine_barrier()

    tile.TileContext._drain_and_barrier = _patched
    tile.TileContext._drain_split_patched = True


# ------------------------------------------------------------- gumbel (CPU)
def _gumbel_cpu():
    """jax.random.gumbel(key(42), (T-1, B, 2), f32) — computed in a CPU-jax
    subprocess so the accelerator backend is never involved (it must be
    bit-identical to the reference's CPU computation)."""
    path = os.path.join(tempfile.mkdtemp(), "gumbel.npy")
    code = (
        "import numpy as np, jax, jax.numpy as jnp\n"
        f"g = jax.random.gumbel(jax.random.key(42), ({T - 1}, {B}, 2), jnp.float32)\n"
        f"np.save({path!r}, np.asarray(g))\n"
    )
    env = dict(os.environ)
    env["TRN_TERMINAL_POOL_IPS"] = ""
    env["JAX_PLATFORMS"] = "cpu"
    extra = [p for p in sys.path if p and os.path.isdir(p)]
    env["PYTHONPATH"] = os.pathsep.join(extra)
    subprocess.run([sys.executable, "-c", code], env=env, check=True, capture_output=True)
    return np.load(path)


# ------------------------------------------------------------- host packing
def _pack_T(a2d, dtype=np.float32):
    """[rows(=128*k), cols] -> [128, k, cols] weight-tile layout."""
    rows, cols = a2d.shape
    k = rows // 128
    return np.ascontiguousarray(a2d.reshape(k, 128, cols).transpose(1, 0, 2)).astype(dtype)


def _pack_bias(b1d):
    """[128*k] -> [128, k]"""
    k = b1d.shape[0] // 128
    return np.ascontiguousarray(b1d.reshape(k, 128).T).astype(np.float32)


def _pack_embT(emb_rows, t_len, dtype=np.float32):
    """[bpc, t_len, E] -> [KE, 128, bpc*t_len] (e-major tiles, free (b, t))."""
    bpc = emb_rows.shape[0]
    x = emb_rows.transpose(2, 0, 1).reshape(KE, 128, bpc * t_len)
    return np.ascontiguousarray(x).astype(dtype)


def _pack_gru_weights(Wih, Whh, bih, bhh, dtype=np.float32):
    """Returns (wihT, whhT, bias_proj, bhhn) packings.

    bias_proj folds bih+bhh for the r,z chunks (added once at projection
    time); n chunks get bih only, with bhh_n applied per-step (it must be
    added to h@Whh_n *before* the r* multiply)."""
    wihT = _pack_T(np.ascontiguousarray(Wih.T), dtype)  # [128, KE|KH, 3H]
    whhT = _pack_T(np.ascontiguousarray(Whh.T), dtype)  # [128, KH, 3H]
    bias = np.empty(3 * H, np.float32)
    bias[: 2 * H] = bih[: 2 * H] + bhh[: 2 * H]
    bias[2 * H :] = bih[2 * H :]
    bias_proj = _pack_bias(bias)      # [128, GC]
    bhhn = _pack_bias(bhh[2 * H :])   # [128, KH]
    return wihT, whhT, bias_proj, bhhn


# ------------------------------------------------------------- bass builders
def _mk_nc():
    import concourse.bass as bass

    return bass.Bass("TRN2", target_bir_lowering=False, debug=False, num_devices=1)


def _split_excess_waits(nc, max_waits=1):
    """This walrus build can only encode ~2 sem waits per instruction
    (setupSyncWait 'Too many sync wait commands'). Hoist excess waits onto
    same-engine NoOps inserted just before the over-subscribed instruction;
    engine queues execute in order, so the wait semantics are identical."""
    from concourse import mybir

    nid = [0]
    for f in nc.m.functions:
        for bb in f.blocks:
            out = []
            changed = False
            for inst in bb.instructions:
                si = inst.sync_info
                lim = max_waits
                if si is not None and si.on_wait and len(si.on_wait) > lim:
                    waits = list(si.on_wait)
                    extra, keep = waits[:-lim], waits[-lim:]
                    for j in range(0, len(extra), max_waits):
                        nop = mybir.InstNoOp(
                            name=f"I-waitnop-{nid[0]}", ins=[], outs=[])
                        nid[0] += 1
                        nop.engine = inst.engine
                        nop.sync_info = mybir.SyncInfo(
                            on_wait=extra[j : j + max_waits], on_update=[])
                        nc.register_instruction(nop, overwrite=True)
                        out.append(nop)
                    inst.sync_info = mybir.SyncInfo(
                        on_wait=keep, on_update=list(si.on_update or []))
                    changed = True
                out.append(inst)
            if changed:
                bb.instructions = out
    return nc


def _emit_proj(nc, mybir, src_dram, wT_sb, bias_sb, gi_tile, kin, t_len,
               dma_pool, ps_pool, src_dtype):
    """gi[c*128+p, t, c, b] = sum_e W[e, c*128+p] * src[e, b, t] + bias.

    src_dram: DRAM [kin, 128, BPC*t_len]; wT_sb: [128, kin, 3H];
    gi_tile: [128, t_len, GC, BPC] (f32)."""
    f32 = mybir.dt.float32
    act = mybir.ActivationFunctionType
    for b in range(BPC):
        src_sb = dma_pool.tile([128, kin, t_len], src_dtype, tag="projsrc")
        for k in range(kin):
            nc.sync.dma_start(
                src_sb[:, k, :], src_dram[k, :, b * t_len : (b + 1) * t_len]
            )
        for c in range(GC):
            ps = ps_pool.tile([128, t_len], f32, tag="projps")
            for k in range(kin):
                nc.tensor.matmul(
                    ps[:],
                    wT_sb[:, k, c * 128 : (c + 1) * 128],
                    src_sb[:, k, :],
                    start=(k == 0),
                    stop=(k == kin - 1),
                )
            nc.scalar.activation(
                gi_tile[:, :, c, b], ps[:], act.Identity, bias=bias_sb[:, c : c + 1]
            )


def _gru_step(nc, mybir, t, h_rhs, gi_tile, whh_sb, bhhn_sb, pools, h_out_ap):
    """Emit one GRU step.

    h_rhs: AP [128, KH, BPC] of h_{t-1} (None for t==0 -> h=0).
    gi_tile[:, t, c, :] holds the projected input (+bias, r/z chunks also
    fold bhh) for step t. Writes h_t to h_out_ap ([128, KH, BPC])."""
    f32 = mybir.dt.float32
    act = mybir.ActivationFunctionType
    alu = mybir.AluOpType

    rz_pool, n_pool, sb_pool = pools

    if h_rhs is not None:
        rz_ps = rz_pool.tile([128, 4, BPC], f32, tag="rzps")
        n_ps = n_pool.tile([128, KH, BPC], f32, tag="nps")
        for c in range(4):
            for k in range(KH):
                nc.tensor.matmul(
                    rz_ps[:, c, :],
                    whh_sb[:, k, c * 128 : (c + 1) * 128],
                    h_rhs[:, k, :],
                    start=(k == 0),
                    stop=(k == KH - 1),
                )
        for c in range(4, GC):
            for k in range(KH):
                nc.tensor.matmul(
                    n_ps[:, c - 4, :],
                    whh_sb[:, k, c * 128 : (c + 1) * 128],
                    h_rhs[:, k, :],
                    start=(k == 0),
                    stop=(k == KH - 1),
                )
        # rz = sigmoid(psum + gi_rz)
        rz = sb_pool.tile([128, 4, BPC], f32, tag="rz")
        nc.vector.tensor_tensor(rz[:], rz_ps[:], gi_tile[:, t, 0:4, :], alu.add)
        nc.scalar.activation(rz[:], rz[:], act.Sigmoid)
        # hn2 = (psum_n + bhh_n) * r  (fused per k chunk), then += gi_n
        hn2 = sb_pool.tile([128, KH, BPC], f32, tag="hn")
        for k in range(KH):
            nc.vector.scalar_tensor_tensor(
                hn2[:, k, :], n_ps[:, k, :], bhhn_sb[:, k : k + 1], rz[:, k, :],
                alu.add, alu.mult,
            )
        nc.vector.tensor_tensor(hn2[:], hn2[:], gi_tile[:, t, 4:GC, :], alu.add)
        nn_ = sb_pool.tile([128, KH, BPC], f32, tag="nn")
        nc.scalar.activation(nn_[:], hn2[:], act.Tanh)
        # h' = n + z * (h - n)
        d = sb_pool.tile([128, KH, BPC], f32, tag="dd")
        nc.vector.tensor_tensor(d[:], h_rhs, nn_[:], alu.subtract)
        nc.vector.tensor_tensor(d[:], rz[:, 2:4, :], d[:], alu.mult)
        nc.vector.tensor_tensor(h_out_ap, nn_[:], d[:], alu.add)
    else:
        # h == 0: rz = sigmoid(gi_rz); n = tanh(gi_n + r*bhh_n); h0 = n - z*n
        rz = sb_pool.tile([128, 4, BPC], f32, tag="rz")
        nc.scalar.activation(rz[:], gi_tile[:, t, 0:4, :], act.Sigmoid)
        hn2 = sb_pool.tile([128, KH, BPC], f32, tag="hn")
        for k in range(KH):
            nc.vector.scalar_tensor_tensor(
                hn2[:, k, :], rz[:, k, :], bhhn_sb[:, k : k + 1],
                gi_tile[:, t, 4 + k, :], alu.mult, alu.add,
            )
        nn_ = sb_pool.tile([128, KH, BPC], f32, tag="nn")
        nc.scalar.activation(nn_[:], hn2[:], act.Tanh)
        d = sb_pool.tile([128, KH, BPC], f32, tag="dd")
        nc.vector.tensor_tensor(d[:], rz[:, 2:4, :], nn_[:], alu.mult)
        nc.vector.tensor_tensor(h_out_ap, nn_[:], d[:], alu.subtract)


def build_kernel1(t_len=T):
    """Select-policy kernel: proj + recurrence + batched decisions."""
    import concourse.tile as tile
    from concourse import mybir

    _apply_tile_patch()
    nc = _mk_nc()
    f32 = mybir.dt.float32
    alu = mybir.AluOpType

    embT_d = nc.dram_tensor("embT", [KE, 128, BPC * t_len], f32, kind="ExternalInput").ap()
    wihcT_d = nc.dram_tensor("wihcT", [128, KE, 3 * H], f32, kind="ExternalInput").ap()
    whhcT_d = nc.dram_tensor("whhcT", [128, KH, 3 * H], f32, kind="ExternalInput").ap()
    wlgT_d = nc.dram_tensor("wlgT", [128, KH, 1], f32, kind="ExternalInput").ap()
    biasc_d = nc.dram_tensor("biasc", [128, GC], f32, kind="ExternalInput").ap()
    bhhnc_d = nc.dram_tensor("bhhnc", [128, KH], f32, kind="ExternalInput").ap()
    ncdiff_d = nc.dram_tensor("ncdiff", [1, t_len * BPC], f32, kind="ExternalInput").ap()
    ks_d = nc.dram_tensor("ks", [1, t_len * BPC], f32, kind="ExternalOutput").ap()

    n_blk = (t_len + LG_BLK - 1) // LG_BLK

    with tile.TileContext(nc) as tc:
        from contextlib import ExitStack

        with ExitStack() as ctx:
            wpool = ctx.enter_context(tc.tile_pool(name="weights", bufs=1))
            gipool = ctx.enter_context(tc.tile_pool(name="gi", bufs=1))
            dma_pool = ctx.enter_context(tc.tile_pool(name="dma", bufs=2))
            big_ps = ctx.enter_context(tc.tile_pool(name="bigps", bufs=2, space="PSUM"))
            rz_pool = ctx.enter_context(tc.tile_pool(name="rzps", bufs=2, space="PSUM"))
            n_pool = ctx.enter_context(tc.tile_pool(name="nps", bufs=2, space="PSUM"))
            sb_pool = ctx.enter_context(tc.tile_pool(name="gates", bufs=2))
            misc = ctx.enter_context(tc.tile_pool(name="misc", bufs=1))

            ncd_pool = ctx.enter_context(tc.tile_pool(name="ncd", bufs=2))
            ks_pool = ctx.enter_context(tc.tile_pool(name="ksp", bufs=2))

            wihcT = wpool.tile([128, KE, 3 * H], f32, tag="wihcT")
            nc.sync.dma_start(wihcT[:], wihcT_d[:])
            whhcT = wpool.tile([128, KH, 3 * H], f32, tag="whhcT")
            nc.sync.dma_start(whhcT[:], whhcT_d[:])
            wlgT = wpool.tile([128, KH, 1], f32, tag="wlgT")
            nc.sync.dma_start(wlgT[:], wlgT_d[:])
            biasc = wpool.tile([128, GC], f32, tag="biasc")
            nc.sync.dma_start(biasc[:], biasc_d[:])
            bhhnc = wpool.tile([128, KH], f32, tag="bhhnc")
            nc.sync.dma_start(bhhnc[:], bhhnc_d[:])

            giT = gipool.tile([128, t_len, GC, BPC], f32, tag="gi")
            _emit_proj(nc, mybir, embT_d, wihcT, biasc, giT, KE, t_len,
                       dma_pool, big_ps, f32)

            h_hist = gipool.tile([128, t_len, KH, BPC], f32, tag="hhist")

            pools = (rz_pool, n_pool, sb_pool)
            h_prev = None
            for t in range(t_len):
                h_out = h_hist[:, t, :, :]
                _gru_step(nc, mybir, t, h_prev, giT, whhcT, bhhnc, pools, h_out)
                h_prev = h_out
                # batched decisions for the finished block
                if (t + 1) % LG_BLK == 0 or t == t_len - 1:
                    t0 = (t // LG_BLK) * LG_BLK
                    nb = (t + 1 - t0) * BPC
                    ncd = ncd_pool.tile([1, LG_BLK * BPC], f32, tag="ncd")
                    nc.sync.dma_start(
                        ncd[0:1, :nb], ncdiff_d[0:1, t0 * BPC : (t + 1) * BPC])
                    lgp = big_ps.tile([1, LG_BLK * BPC], f32, tag="lgps")
                    for k in range(KH):
                        nc.tensor.matmul(
                            lgp[:, :nb],
                            wlgT[:, k, :],
                            h_hist[:, t0 : t + 1, k, :],
                            start=(k == 0),
                            stop=(k == KH - 1),
                        )
                    ks_sb = ks_pool.tile([1, LG_BLK * BPC], f32, tag="kssb")
                    nc.vector.tensor_tensor(
                        ks_sb[0:1, :nb], lgp[:, :nb], ncd[0:1, :nb], alu.is_gt)
                    nc.sync.dma_start(
                        ks_d[0:1, t0 * BPC : (t + 1) * BPC], ks_sb[0:1, :nb])

    return _split_excess_waits(nc)


def build_kernel2(t_len):
    """GRU0/GRU1 + convs + pooling + final linear (bf16 matmuls)."""
    import concourse.tile as tile
    from concourse import mybir

    _apply_tile_patch()
    nc = _mk_nc()
    f32 = mybir.dt.float32
    bf16 = mybir.dt.bfloat16
    act = mybir.ActivationFunctionType
    alu = mybir.AluOpType

    nembT_d = nc.dram_tensor("nembT", [KE, 128, BPC * t_len], bf16, kind="ExternalInput").ap()
    wih0T_d = nc.dram_tensor("wih0T", [128, KE, 3 * H], bf16, kind="ExternalInput").ap()
    whh0T_d = nc.dram_tensor("whh0T", [128, KH, 3 * H], bf16, kind="ExternalInput").ap()
    bias0_d = nc.dram_tensor("bias0", [128, GC], f32, kind="ExternalInput").ap()
    bhhn0_d = nc.dram_tensor("bhhn0", [128, KH], f32, kind="ExternalInput").ap()
    wih1T_d = nc.dram_tensor("wih1T", [128, KH, 3 * H], bf16, kind="ExternalInput").ap()
    whh1T_d = nc.dram_tensor("whh1T", [128, KH, 3 * H], bf16, kind="ExternalInput").ap()
    bias1_d = nc.dram_tensor("bias1", [128, GC], f32, kind="ExternalInput").ap()
    bhhn1_d = nc.dram_tensor("bhhn1", [128, KH], f32, kind="ExternalInput").ap()
    cw_d = nc.dram_tensor("cw", [128, 12, KH, NF], bf16, kind="ExternalInput").ap()
    cb_d = nc.dram_tensor("cb", [NF, 3], f32, kind="ExternalInput").ap()
    vt_d = nc.dram_tensor("vt", [1, BPC * t_len], f32, kind="ExternalInput").ap()
    tmask_d = nc.dram_tensor("tmask", [NF, 3, t_len], f32, kind="ExternalInput").ap()
    woutT_d = nc.dram_tensor("woutT", [NF, 3], f32, kind="ExternalInput").ap()
    bout_d = nc.dram_tensor("bout", [1, 1], f32, kind="ExternalInput").ap()
    out_d = nc.dram_tensor("out", [1, BPC], f32, kind="ExternalOutput").ap()

    FS = (3, 4, 5)

    with tile.TileContext(nc) as tc:
        from contextlib import ExitStack

        with ExitStack() as ctx:
            wpool = ctx.enter_context(tc.tile_pool(name="weights", bufs=1))
            gipool = ctx.enter_context(tc.tile_pool(name="gi", bufs=1))
            opool = ctx.enter_context(tc.tile_pool(name="obuf", bufs=1))
            dma_pool = ctx.enter_context(tc.tile_pool(name="dma", bufs=2))
            big_ps = ctx.enter_context(tc.tile_pool(name="bigps", bufs=2, space="PSUM"))
            rz_pool = ctx.enter_context(tc.tile_pool(name="rzps", bufs=2, space="PSUM"))
            n_pool = ctx.enter_context(tc.tile_pool(name="nps", bufs=2, space="PSUM"))
            fin_ps = ctx.enter_context(tc.tile_pool(name="finps", bufs=1, space="PSUM"))
            sb_pool = ctx.enter_context(tc.tile_pool(name="gates", bufs=2))
            misc = ctx.enter_context(tc.tile_pool(name="misc", bufs=1))

            def _load(pool, dram, shape, tag, dt_):
                t_ = pool.tile(shape, dt_, tag=tag)
                nc.sync.dma_start(t_[:], dram[:])
                return t_

            wih0T = _load(wpool, wih0T_d, [128, KE, 3 * H], "wih0T", bf16)
            whh0T = _load(wpool, whh0T_d, [128, KH, 3 * H], "whh0T", bf16)
            wih1T = _load(wpool, wih1T_d, [128, KH, 3 * H], "wih1T", bf16)
            whh1T = _load(wpool, whh1T_d, [128, KH, 3 * H], "whh1T", bf16)
            cw = _load(wpool, cw_d, [128, 12, KH, NF], "cw", bf16)
            bias0 = _load(wpool, bias0_d, [128, GC], "bias0", f32)
            bhhn0 = _load(wpool, bhhn0_d, [128, KH], "bhhn0", f32)
            bias1 = _load(wpool, bias1_d, [128, GC], "bias1", f32)
            bhhn1 = _load(wpool, bhhn1_d, [128, KH], "bhhn1", f32)
            cb = _load(misc, cb_d, [NF, 3], "cb", f32)
            tmask = _load(misc, tmask_d, [NF, 3, t_len], "tmask", f32)
            woutT = _load(misc, woutT_d, [NF, 3], "woutT", f32)
            bout = _load(misc, bout_d, [1, 1], "bout", f32)
            vt = _load(misc, vt_d, [1, BPC * t_len], "vt", f32)

            pools = (rz_pool, n_pool, sb_pool)

            # ---- layer 0 ----
            gi0 = gipool.tile([128, t_len, GC, BPC], f32, tag="gi")
            _emit_proj(nc, mybir, nembT_d, wih0T, bias0, gi0, KE, t_len,
                       dma_pool, big_ps, bf16)
            o1 = opool.tile([128, KH, BPC, t_len], bf16, tag="o1")
            h_prev = None
            for t in range(t_len):
                _gru_step(nc, mybir, t, h_prev, gi0, whh0T, bhhn0, pools,
                          o1[:, :, :, t])
                h_prev = o1[:, :, :, t]

            # ---- proj1: gi1 = (o1 @ Wih1.T) + bias1 ----
            gi1 = gipool.tile([128, t_len, GC, BPC], f32, tag="gi")
            for b in range(BPC):
                for c in range(GC):
                    ps = big_ps.tile([128, t_len], f32, tag="projps")
                    for k in range(KH):
                        nc.tensor.matmul(
                            ps[:],
                            wih1T[:, k, c * 128 : (c + 1) * 128],
                            o1[:, k, b, :],
                            start=(k == 0),
                            stop=(k == KH - 1),
                        )
                    nc.scalar.activation(
                        gi1[:, :, c, b], ps[:], act.Identity, bias=bias1[:, c : c + 1]
                    )

            # ---- layer 1 ----
            o2 = opool.tile([128, KH, BPC, t_len], bf16, tag="o2")
            h_prev = None
            for t in range(t_len):
                _gru_step(nc, mybir, t, h_prev, gi1, whh1T, bhhn1, pools,
                          o2[:, :, :, t])
                h_prev = o2[:, :, :, t]

            # ---- zero o2 past new_lens (o2 *= vt) ----
            # partition-broadcast vt via a K=1 ones-matmul (PE outer product);
            # windows straddling the sequence end see zero-padded values, so
            # the vt-zeroing must happen before the convs.
            ones_sb = misc.tile([1, 128], f32, tag="ones")
            nc.vector.memset(ones_sb[:], 1.0)
            for b in range(BPC):
                vtb = big_ps.tile([128, t_len], f32, tag="projps")
                nc.tensor.matmul(
                    vtb[:], ones_sb[:], vt[0:1, b * t_len : (b + 1) * t_len],
                    start=True, stop=True,
                )
                for k in range(KH):
                    nc.vector.tensor_tensor(
                        o2[:, k, b, :], o2[:, k, b, :], vtb[:], alu.mult
                    )

            # ---- convs + relu + Ldyn window mask + max-pool ----
            pooled = misc.tile([NF, 3, BPC], f32, tag="pooled")
            for b in range(BPC):
                for fi, fs in enumerate(FS):
                    nw = t_len - fs + 1
                    ps = big_ps.tile([NF, t_len], f32, tag="projps")
                    m0 = sum(FS[:fi])  # flat (fs,dt) base index
                    first = True
                    for dt in range(fs):
                        for k in range(KH):
                            nc.tensor.matmul(
                                ps[:, :nw],
                                cw[:, m0 + dt, k, :],
                                o2[:, k, b, dt : dt + nw],
                                start=first,
                                stop=(dt == fs - 1 and k == KH - 1),
                            )
                            first = False
                    crelu = sb_pool.tile([NF, t_len], f32, tag="crelu")
                    nc.scalar.activation(
                        crelu[:, :nw], ps[:, :nw], act.Relu, bias=cb[:, fi : fi + 1]
                    )
                    nc.vector.tensor_tensor(
                        crelu[:, :nw], crelu[:, :nw], tmask[:, fi, :nw], alu.add
                    )
                    nc.vector.tensor_reduce(
                        pooled[:, fi, b : b + 1], crelu[:, :nw],
                        mybir.AxisListType.X, alu.max,
                    )

            # ---- final linear ----
            fps = fin_ps.tile([1, BPC], f32)
            for fi in range(3):
                nc.tensor.matmul(
                    fps[:],
                    woutT[:, fi : fi + 1],
                    pooled[:, fi, :],
                    start=(fi == 0),
                    stop=(fi == 2),
                )
            out_sb = misc.tile([1, BPC], f32, tag="outsb")
            nc.scalar.activation(out_sb[:], fps[:], act.Identity, bias=bout[0:1, 0:1])
            nc.sync.dma_start(out_d[:], out_sb[:])

    return _split_excess_waits(nc)


# ------------------------------------------------------------- host orchestration
def _host_pack_k1(inputs, gumbel, t_len=T):
    emb = np.asarray(inputs["embedded"], np.float32)
    mask = np.asarray(inputs["mask"])
    lens = mask.sum(1)
    maxlen = int(lens.max())

    wihcT, whhcT, biasc, bhhnc = _pack_gru_weights(
        inputs["Wih_c"], inputs["Whh_c"], inputs["bih_c"], inputs["bhh_c"])
    wlgT = _pack_T(
        np.ascontiguousarray((inputs["Wsel"][1] - inputs["Wsel"][0])[:, None]))
    bdiff = float(inputs["bsel"][1] - inputs["bsel"][0])

    # ncdiff[t, b]: k_t = (h.wdiff > ncdiff); forced to 0 when t >= maxlen-1
    ncdiff = np.full((t_len, B), 1.0e30, np.float32)
    upto = min(maxlen - 1, t_len)
    for t in range(1, upto):
        ncdiff[t] = -(bdiff + gumbel[t - 1, :, 1] - gumbel[t - 1, :, 0])

    in_maps = []
    for c in range(NCORES):
        rows = slice(c * BPC, (c + 1) * BPC)
        in_maps.append({
            "embT": _pack_embT(emb[rows, :t_len], t_len),
            "wihcT": wihcT,
            "whhcT": whhcT,
            "wlgT": wlgT,
            "biasc": biasc,
            "bhhnc": bhhnc,
            "ncdiff": np.ascontiguousarray(ncdiff[:, rows]).reshape(1, t_len * BPC),
        })
    return in_maps, lens, maxlen


def _host_compact(inputs, ks_full, lens, maxlen, t_len=T):
    """ks_full: [B, t_len] decision bits (row t=0 ignored; selected[:,0]=1)."""
    emb = np.asarray(inputs["embedded"], np.float32)
    selected = np.zeros((B, t_len), np.int64)
    selected[:, 0] = 1
    selected[:, 1:] = ks_full[:, 1:]
    pos = np.arange(t_len)
    sel_valid = np.where(pos[None, :] < (lens - 1)[:, None], selected, 0)
    new_mask = np.where(pos[None, :] == (lens - 1)[:, None], 1, sel_valid)
    new_lens = new_mask.sum(1)
    Ldyn = max(int(new_lens.max()), 7)

    new_emb = np.zeros((B, t_len, E), np.float32)
    for b in range(B):
        idx = np.nonzero(new_mask[b])[0]
        new_emb[b, : len(idx)] = emb[b, idx]
    return new_emb, new_lens, Ldyn


def _host_pack_k2(inputs, new_emb, new_lens, Ldyn, t_len):
    wih0T, whh0T, bias0, bhhn0 = _pack_gru_weights(
        inputs["Wih0"], inputs["Whh0"], inputs["bih0"], inputs["bhh0"], BF16)
    wih1T, whh1T, bias1, bhhn1 = _pack_gru_weights(
        inputs["Wih1"], inputs["Whh1"], inputs["bih1"], inputs["bhh1"], BF16)

    FS = (3, 4, 5)
    cw = np.zeros((128, 12, KH, NF), BF16)
    cb = np.zeros((NF, 3), np.float32)
    m = 0
    for fi, fs in enumerate(FS):
        w = np.asarray(inputs[f"conv_w{fs}"], np.float32)  # [NF,1,fs,H]
        cb[:, fi] = np.asarray(inputs[f"conv_b{fs}"], np.float32)
        for dt in range(fs):
            wt = w[:, 0, dt, :].T  # [H, NF]
            cw[:, m, :, :] = wt.reshape(KH, 128, NF).transpose(1, 0, 2).astype(BF16)
            m += 1

    tmask = np.full((NF, 3, t_len), NEG, np.float32)
    for fi, fs in enumerate(FS):
        kf = min(Ldyn - fs + 1, t_len - fs + 1)
        if kf > 0:
            tmask[:, fi, :kf] = 0.0

    vt_full = (np.arange(t_len)[None, :] < new_lens[:, None]).astype(np.float32)

    woutT = np.ascontiguousarray(
        np.asarray(inputs["Wout"], np.float32)[0].reshape(3, NF).T)
    bout = np.asarray(inputs["bout"], np.float32).reshape(1, 1)

    in_maps = []
    for c in range(NCORES):
        rows = slice(c * BPC, (c + 1) * BPC)
        in_maps.append({
            "nembT": _pack_embT(new_emb[rows, :t_len], t_len, BF16),
            "wih0T": wih0T, "whh0T": whh0T, "bias0": bias0, "bhhn0": bhhn0,
            "wih1T": wih1T, "whh1T": whh1T, "bias1": bias1, "bhhn1": bhhn1,
            "cw": cw, "cb": cb, "tmask": tmask,
            "vt": np.ascontiguousarray(vt_full[rows].reshape(1, BPC * t_len)),
            "woutT": woutT, "bout": bout,
        })
    return in_maps


_NC_CACHE = {}


def _get_nc(which, t_len):
    key = (which, t_len)
    if key not in _NC_CACHE:
        _NC_CACHE[key] = build_kernel1(t_len) if which == 1 else build_kernel2(t_len)
    return _NC_CACHE[key]


TRACE = False  # set True (with an NTFF hook registered) to collect exec times
LAST_STATS = {}


def kernel(**inputs):
    from concourse import bass_utils

    gumbel = _gumbel_cpu()
    core_ids = list(range(NCORES))

    in_maps1, lens, maxlen = _host_pack_k1(inputs, gumbel)
    nc1 = _get_nc(1, T)
    res1 = bass_utils.run_bass_kernel_spmd(nc1, in_maps1, core_ids, trace=TRACE)
    ks_full = np.concatenate(
        [res1.results[c]["ks"].reshape(T, BPC).T for c in range(NCORES)], axis=0)

    new_emb, new_lens, Ldyn = _host_compact(inputs, ks_full, lens, maxlen)
    TL = min(T, max(8 * ((Ldyn + 7) // 8), 16))
    in_maps2 = _host_pack_k2(inputs, new_emb, new_lens, Ldyn, TL)
    nc2 = _get_nc(2, TL)
    res2 = bass_utils.run_bass_kernel_spmd(nc2, in_maps2, core_ids, trace=TRACE)
    out = np.concatenate([res2.results[c]["out"][0] for c in range(NCORES)], axis=0)
    LAST_STATS["k1_ns"] = res1.exec_time_ns
    LAST_STATS["k2_ns"] = res2.exec_time_ns
    LAST_STATS["ks"] = ks_full
    LAST_STATS["new_lens"] = new_lens
    LAST_STATS["TL"] = TL
    return out.astype(np.float32)
